# revision 2
# baseline (speedup 1.0000x reference)
"""Evoformer block (single-seq) on 8 Trainium2 NeuronCores.

Strategy (FastFold DAP style, per sharding_hint): shard the pair tensor z
along the first L axis across the 8 cores; s-track + triangle ops computed
with all-gathers of the contracted L axis where needed; final gather of the
row shards assembles the full output.

kernel(**inputs) takes FULL inputs, returns the FULL output tuple (s, z).
"""

import functools
import numpy as np
import jax
import jax.numpy as jnp
from jax.sharding import Mesh, PartitionSpec as P
from jax.experimental.shard_map import shard_map

C_S = 384; C_Z = 128; CH_SA = 32; CH_OPM = 32; CH_MUL = 128; CH_PA = 32
H_S = 12; H_P = 8; N = 1; L = 192
EPS = 1e-5
NCORES = 8
RS = L // NCORES  # 24 rows per core


def _ln(x, g, b):
    mu = jnp.mean(x, -1, keepdims=True)
    var = jnp.mean((x - mu) ** 2, -1, keepdims=True)
    return (x - mu) * jax.lax.rsqrt(var + EPS) * g + b


def _mha_rows(xq, xkv, p, bias, H):
    """Gated MHA: queries from xq rows, keys/values from xkv (full length).

    xq: [..., Lq, c], xkv: [..., Lk, c], bias broadcastable to [..., H, Lq, Lk].
    """
    ch = p['wq'].shape[1] // H
    shq = xq.shape[:-1] + (H, ch)
    shk = xkv.shape[:-1] + (H, ch)
    q = (xq @ p['wq']).reshape(shq) * (ch ** -0.5)
    k = (xkv @ p['wk']).reshape(shk)
    v = (xkv @ p['wv']).reshape(shk)
    a = jax.nn.softmax(jnp.einsum('...qhc,...khc->...hqk', q, k) + bias, axis=-1)
    o = jnp.einsum('...hqk,...khc->...qhc', a, v)
    g = jax.nn.sigmoid(xq @ p['wg'] + p['bg']).reshape(shq)
    o = (o * g).reshape(xq.shape[:-1] + (H * ch,))
    return o @ p['wo'] + p['bo']


def _transition(x, p):
    x = _ln(x, p['ln_g'], p['ln_b'])
    return jax.nn.relu(x @ p['w1'] + p['b1']) @ p['w2'] + p['b2']


def _evoformer_shard(s, z, p):
    """Runs on one core. s: [1, L, C_S] replicated; z: [1, RS, L, C_Z] row shard.

    Returns (s_shard [1, RS, C_S], z_shard [1, RS, L, C_Z]).
    """
    ax = 'x'

    # ---- s track ----
    sn = _ln(s, p['sa']['ln_s_g'], p['sa']['ln_s_b'])
    znb = _ln(z, p['sa']['ln_z_g'], p['sa']['ln_z_b'])
    # bias rows for our z shard -> all-gather to full [N, H_S, L, L]
    bias_sh = jnp.transpose(znb @ p['sa']['w_bias'], (0, 3, 1, 2))  # [N,H,RS,L]
    bias = jax.lax.all_gather(bias_sh, ax, axis=2, tiled=True)      # [N,H,L,L]
    # each core computes attention for its RS query rows of s
    i0 = jax.lax.axis_index(ax) * RS
    sn_q = jax.lax.dynamic_slice_in_dim(sn, i0, RS, axis=1)
    s_q = jax.lax.dynamic_slice_in_dim(s, i0, RS, axis=1)
    bias_q = jax.lax.dynamic_slice_in_dim(bias, i0, RS, axis=2)
    s_row = s_q + _mha_rows(sn_q, sn, p['sa'], bias_q, H_S)         # [N,RS,C_S]
    s_row = s_row + _transition(s_row, p['st'])
    # full s needed for opm bf over all j
    s_full = jax.lax.all_gather(s_row, ax, axis=1, tiled=True)      # [N,L,C_S]

    # ---- outer product mean ----
    snf = _ln(s_full, p['opm']['ln_g'], p['opm']['ln_b'])
    af = jax.lax.dynamic_slice_in_dim(snf, i0, RS, axis=1) @ p['opm']['w1'] + p['opm']['b1']
    bf = snf @ p['opm']['w2'] + p['opm']['b2']                      # [N,L,ch]
    o = jnp.einsum('nia,njb->nijab', af, bf).reshape(z.shape[0], RS, L, -1)
    z = z + o @ p['opm']['wo'] + p['opm']['bo']

    # ---- triangle mult outgoing: x[i,j,:] = sum_k a[i,k] b[j,k] ----
    pp = p['tmo']
    zn = _ln(z, pp['ln_in_g'], pp['ln_in_b'])
    a = jax.nn.sigmoid(zn @ pp['w_ag'] + pp['b_ag']) * (zn @ pp['w_ap'] + pp['b_ap'])
    b = jax.nn.sigmoid(zn @ pp['w_bg'] + pp['b_bg']) * (zn @ pp['w_bp'] + pp['b_bp'])
    b_full = jax.lax.all_gather(b, ax, axis=1, tiled=True)          # [N,L,L,C]
    x = jnp.einsum('nikc,njkc->nijc', a, b_full)
    x = _ln(x, pp['ln_out_g'], pp['ln_out_b']) @ pp['w_z'] + pp['b_z']
    z = z + jax.nn.sigmoid(zn @ pp['w_g'] + pp['b_g']) * x

    # ---- triangle mult incoming: x[i,j,:] = sum_k a[k,i] b[k,j] ----
    pp = p['tmi']
    zn = _ln(z, pp['ln_in_g'], pp['ln_in_b'])
    a = jax.nn.sigmoid(zn @ pp['w_ag'] + pp['b_ag']) * (zn @ pp['w_ap'] + pp['b_ap'])
    b = jax.nn.sigmoid(zn @ pp['w_bg'] + pp['b_bg']) * (zn @ pp['w_bp'] + pp['b_bp'])
    a_full = jax.lax.all_gather(a, ax, axis=1, tiled=True)
    b_full = jax.lax.all_gather(b, ax, axis=1, tiled=True)
    a_i = jax.lax.dynamic_slice_in_dim(a_full, i0, RS, axis=2)      # [N,L,RS,C]
    x = jnp.einsum('nkic,nkjc->nijc', a_i, b_full)
    x = _ln(x, pp['ln_out_g'], pp['ln_out_b']) @ pp['w_z'] + pp['b_z']
    z = z + jax.nn.sigmoid(zn @ pp['w_g'] + pp['b_g']) * x

    # ---- triangle attention starting (rows attend within row) ----
    pp = p['tas']
    zn = _ln(z, pp['ln_g'], pp['ln_b'])
    bias_sh = jnp.transpose(zn @ pp['w_bias'], (0, 3, 1, 2))        # [N,H,RS,L]
    bias = jax.lax.all_gather(bias_sh, ax, axis=2, tiled=True)[:, None]  # [N,1,H,L,L]
    z = z + _mha_rows(zn, zn, pp, bias, H_P)

    # ---- triangle attention ending (columns attend within column) ----
    # Swapped frame: batch = columns j (all 192), sequence = rows. We compute
    # only the output rows in our shard (queries = our rows, keys = all rows).
    pp = p['tae']
    zn = _ln(z, pp['ln_g'], pp['ln_b'])
    zn_full = jax.lax.all_gather(zn, ax, axis=1, tiled=True)        # [N,L(row),L(col),C]
    znT_full = jnp.swapaxes(zn_full, 1, 2)                          # [N,col,row,C]
    # bias'[h, q, k] = proj(zn[row=k, col=q]); from full row-major bias map
    bias_sh = jnp.transpose(zn @ pp['w_bias'], (0, 3, 1, 2))        # [N,H,RS(row),L(col)]
    bias_full = jax.lax.all_gather(bias_sh, ax, axis=2, tiled=True) # [N,H,row,col]
    biasT = jnp.swapaxes(bias_full, 2, 3)[:, None]                  # [N,1,H,q(col)?,k] -> see note
    # note: biasT[n,0,h,a,b] = bias_full[n,h,row=b,col=a] = proj(zn[b, a]),
    # matching score[..., q=a(row-swapped), k=b]: bias'[h,q,k] = proj(zn'[q,k]).
    znT_q = jax.lax.dynamic_slice_in_dim(znT_full, i0, RS, axis=2)  # [N,col,RS(our rows),C]
    biasT_q = jax.lax.dynamic_slice_in_dim(biasT, i0, RS, axis=3)   # [N,1,H,RS,L]
    o = _mha_rows(znT_q, znT_full, pp, biasT_q, H_P)                # [N,col,RS,C]
    z = z + jnp.swapaxes(o, 1, 2)                                   # [N,RS,col,C]

    # ---- pair transition ----
    z = z + _transition(z, p['pt'])
    return s_row, z


def _build_sharded(params):
    mesh = Mesh(np.array(jax.devices()[:NCORES]), ('x',))

    @functools.partial(shard_map, mesh=mesh,
                       in_specs=(P(), P(None, 'x', None, None)),
                       out_specs=(P(None, 'x', None), P(None, 'x', None, None)),
                       check_rep=False)
    def f(s, z):
        return _evoformer_shard(s, z, params)

    return jax.jit(f)


def kernel(s, z, params):
    s = jnp.asarray(s); z = jnp.asarray(z)
    f = _build_sharded(params)
    s_out, z_out = f(s, z)
    return np.asarray(s_out), np.asarray(z_out)


# revision 5
# speedup vs baseline: 1.3030x; 1.3030x over previous
"""Evoformer block (single-seq) on 8 Trainium2 NeuronCores.

Strategy (FastFold DAP style, per sharding_hint): shard the pair tensor z
along the first L axis across the 8 cores; s-track + triangle ops computed
with all-gathers of the contracted L axis where needed; final gather of the
row shards assembles the full output.

kernel(**inputs) takes FULL inputs, returns the FULL output tuple (s, z).
"""

import functools
import numpy as np
import jax
import jax.numpy as jnp
from jax.sharding import Mesh, PartitionSpec as P
from jax.experimental.shard_map import shard_map

C_S = 384; C_Z = 128; CH_SA = 32; CH_OPM = 32; CH_MUL = 128; CH_PA = 32
H_S = 12; H_P = 8; N = 1; L = 192
EPS = 1e-5
NCORES = 8
RS = L // NCORES  # 24 rows per core


def _ln(x, g, b):
    mu = jnp.mean(x, -1, keepdims=True)
    var = jnp.mean((x - mu) ** 2, -1, keepdims=True)
    return (x - mu) * jax.lax.rsqrt(var + EPS) * g + b


def _mha_rows(xq, xkv, p, bias, H):
    """Gated MHA: queries from xq rows, keys/values from xkv (full length).

    xq: [..., Lq, c], xkv: [..., Lk, c], bias broadcastable to [..., H, Lq, Lk].
    """
    ch = p['wq'].shape[1] // H
    shq = xq.shape[:-1] + (H, ch)
    shk = xkv.shape[:-1] + (H, ch)
    q = (xq @ p['wq']).reshape(shq) * (ch ** -0.5)
    k = (xkv @ p['wk']).reshape(shk)
    v = (xkv @ p['wv']).reshape(shk)
    a = jax.nn.softmax(jnp.einsum('...qhc,...khc->...hqk', q, k) + bias, axis=-1)
    o = jnp.einsum('...hqk,...khc->...qhc', a, v)
    g = jax.nn.sigmoid(xq @ p['wg'] + p['bg']).reshape(shq)
    o = (o * g).reshape(xq.shape[:-1] + (H * ch,))
    return o @ p['wo'] + p['bo']


def _transition(x, p):
    x = _ln(x, p['ln_g'], p['ln_b'])
    return jax.nn.relu(x @ p['w1'] + p['b1']) @ p['w2'] + p['b2']


def _evoformer_shard(s, z, p):
    """Runs on one core. s: [1, L, C_S] replicated; z: [1, RS, L, C_Z] row shard.

    Returns (s_shard [1, RS, C_S], z_shard [1, RS, L, C_Z]).
    """
    ax = 'x'

    # ---- s track ----
    sn = _ln(s, p['sa']['ln_s_g'], p['sa']['ln_s_b'])
    znb = _ln(z, p['sa']['ln_z_g'], p['sa']['ln_z_b'])
    # bias rows for our z shard -> all-gather to full [N, H_S, L, L]
    bias_sh = jnp.transpose(znb @ p['sa']['w_bias'], (0, 3, 1, 2))  # [N,H,RS,L]
    bias = jax.lax.all_gather(bias_sh, ax, axis=2, tiled=True)      # [N,H,L,L]
    # each core computes attention for its RS query rows of s
    i0 = jax.lax.axis_index(ax) * RS
    sn_q = jax.lax.dynamic_slice_in_dim(sn, i0, RS, axis=1)
    s_q = jax.lax.dynamic_slice_in_dim(s, i0, RS, axis=1)
    bias_q = jax.lax.dynamic_slice_in_dim(bias, i0, RS, axis=2)
    s_row = s_q + _mha_rows(sn_q, sn, p['sa'], bias_q, H_S)         # [N,RS,C_S]
    s_row = s_row + _transition(s_row, p['st'])
    # full s needed for opm bf over all j
    s_full = jax.lax.all_gather(s_row, ax, axis=1, tiled=True)      # [N,L,C_S]

    # ---- outer product mean ----
    snf = _ln(s_full, p['opm']['ln_g'], p['opm']['ln_b'])
    af = jax.lax.dynamic_slice_in_dim(snf, i0, RS, axis=1) @ p['opm']['w1'] + p['opm']['b1']
    bf = snf @ p['opm']['w2'] + p['opm']['b2']                      # [N,L,ch]
    # factored: (af x bf) . wo == af . (bf . wo_abc)  -- rank-32, ~30x fewer MACs
    wo3 = p['opm']['wo'].reshape(CH_OPM, CH_OPM, C_Z)               # [a,b,c]
    V = jnp.einsum('nia,abc->nibc', af, wo3)                        # [N,RS,b,C]
    z = z + jnp.einsum('njb,nibc->nijc', bf, V) + p['opm']['bo']

    # ---- triangle mult outgoing: x[i,j,:] = sum_k a[i,k] b[j,k] ----
    pp = p['tmo']
    zn = _ln(z, pp['ln_in_g'], pp['ln_in_b'])
    a = jax.nn.sigmoid(zn @ pp['w_ag'] + pp['b_ag']) * (zn @ pp['w_ap'] + pp['b_ap'])
    b = jax.nn.sigmoid(zn @ pp['w_bg'] + pp['b_bg']) * (zn @ pp['w_bp'] + pp['b_bp'])
    b_full = jax.lax.all_gather(b, ax, axis=1, tiled=True)          # [N,L,L,C]
    x = jnp.einsum('nikc,njkc->nijc', a, b_full)
    x = _ln(x, pp['ln_out_g'], pp['ln_out_b']) @ pp['w_z'] + pp['b_z']
    z = z + jax.nn.sigmoid(zn @ pp['w_g'] + pp['b_g']) * x

    # ---- triangle mult incoming: x[i,j,:] = sum_k a[k,i] b[k,j] ----
    pp = p['tmi']
    zn = _ln(z, pp['ln_in_g'], pp['ln_in_b'])
    a = jax.nn.sigmoid(zn @ pp['w_ag'] + pp['b_ag']) * (zn @ pp['w_ap'] + pp['b_ap'])
    b = jax.nn.sigmoid(zn @ pp['w_bg'] + pp['b_bg']) * (zn @ pp['w_bp'] + pp['b_bp'])
    a_full = jax.lax.all_gather(a, ax, axis=1, tiled=True)
    b_full = jax.lax.all_gather(b, ax, axis=1, tiled=True)
    a_i = jax.lax.dynamic_slice_in_dim(a_full, i0, RS, axis=2)      # [N,L,RS,C]
    x = jnp.einsum('nkic,nkjc->nijc', a_i, b_full)
    x = _ln(x, pp['ln_out_g'], pp['ln_out_b']) @ pp['w_z'] + pp['b_z']
    z = z + jax.nn.sigmoid(zn @ pp['w_g'] + pp['b_g']) * x

    # ---- triangle attention starting (rows attend within row) ----
    # einsum-only form: batch (n, i=row), q/k = columns; no big transposes.
    pp = p['tas']
    zn = _ln(z, pp['ln_g'], pp['ln_b'])
    ch = CH_PA
    sh = (z.shape[0], RS, L, H_P, ch)
    q = (zn @ pp['wq']).reshape(sh) * (ch ** -0.5)
    k = (zn @ pp['wk']).reshape(sh)
    v = (zn @ pp['wv']).reshape(sh)
    pb_loc = zn @ pp['w_bias']                                      # [N,RS,L,H]
    pb = jax.lax.all_gather(pb_loc, ax, axis=1, tiled=True)         # [N,L(q),L(k),H]
    bias = jnp.transpose(pb, (0, 1, 3, 2))[:, None]                 # [N,1,q,H,k]
    sc = jnp.einsum('niqhc,nikhc->niqhk', q, k) + bias              # [N,i,q,H,k]
    a = jax.nn.softmax(sc, axis=-1)
    o = jnp.einsum('niqhk,nikhc->niqhc', a, v)
    g = jax.nn.sigmoid(zn @ pp['wg'] + pp['bg']).reshape(sh)
    o = (o * g).reshape(z.shape[0], RS, L, H_P * ch)
    z = z + o @ pp['wo'] + pp['bo']

    # ---- triangle attention ending (columns attend within column) ----
    # einsum-only: batch (n, j=col), queries = our rows (local zn), keys = all
    # rows (gathered zn). Output lands row-major directly; no big transposes.
    pp = p['tae']
    zn = _ln(z, pp['ln_g'], pp['ln_b'])
    zn_full = jax.lax.all_gather(zn, ax, axis=1, tiled=True)        # [N,L(row),L(col),C]
    ch = CH_PA
    shq = (z.shape[0], RS, L, H_P, ch)
    shk = (z.shape[0], L, L, H_P, ch)
    q = (zn @ pp['wq']).reshape(shq) * (ch ** -0.5)                 # [N,q(row),j,H,c]
    k = (zn_full @ pp['wk']).reshape(shk)                           # [N,k(row),j,H,c]
    v = (zn_full @ pp['wv']).reshape(shk)
    # bias'[h, q, k] = proj(zn[row=k, col=q_global]): slice cols to our rows
    pb = zn_full @ pp['w_bias']                                     # [N,k(row),col,H]
    pb_q = jax.lax.dynamic_slice_in_dim(pb, i0, RS, axis=2)         # [N,k,RS(q),H]
    bias = jnp.transpose(pb_q, (0, 2, 3, 1))[:, :, None]            # [N,q,1,H,k]
    sc = jnp.einsum('nqjhc,nkjhc->nqjhk', q, k) + bias              # [N,q,j,H,k]
    a = jax.nn.softmax(sc, axis=-1)
    o = jnp.einsum('nqjhk,nkjhc->nqjhc', a, v)                      # [N,q(row),j,H,c]
    g = jax.nn.sigmoid(zn @ pp['wg'] + pp['bg']).reshape(shq)
    o = (o * g).reshape(z.shape[0], RS, L, H_P * ch)
    z = z + o @ pp['wo'] + pp['bo']

    # ---- pair transition ----
    z = z + _transition(z, p['pt'])
    return s_row, z


def _build_sharded(params):
    mesh = Mesh(np.array(jax.devices()[:NCORES]), ('x',))

    @functools.partial(shard_map, mesh=mesh,
                       in_specs=(P(), P(None, 'x', None, None)),
                       out_specs=(P(None, 'x', None), P(None, 'x', None, None)),
                       check_rep=False)
    def f(s, z):
        return _evoformer_shard(s, z, params)

    return jax.jit(f)


def kernel(s, z, params):
    s = jnp.asarray(s); z = jnp.asarray(z)
    f = _build_sharded(params)
    s_out, z_out = f(s, z)
    return np.asarray(s_out), np.asarray(z_out)


# revision 10
# speedup vs baseline: 1.3117x; 1.0067x over previous
"""Evoformer block (single-seq) on 8 Trainium2 NeuronCores.

Strategy (FastFold DAP style, per sharding_hint): shard the pair tensor z
along the first L axis across the 8 cores; s-track + triangle ops computed
with all-gathers of the contracted L axis where needed; final gather of the
row shards assembles the full output.

kernel(**inputs) takes FULL inputs, returns the FULL output tuple (s, z).
"""

import functools
import numpy as np
import jax
import jax.numpy as jnp
from jax.sharding import Mesh, PartitionSpec as P
from jax.experimental.shard_map import shard_map

C_S = 384; C_Z = 128; CH_SA = 32; CH_OPM = 32; CH_MUL = 128; CH_PA = 32
H_S = 12; H_P = 8; N = 1; L = 192
EPS = 1e-5
NCORES = 8
RS = L // NCORES  # 24 rows per core


def _ln(x, g, b):
    mu = jnp.mean(x, -1, keepdims=True)
    var = jnp.mean((x - mu) ** 2, -1, keepdims=True)
    return (x - mu) * jax.lax.rsqrt(var + EPS) * g + b


def _mha_rows(xq, xkv, p, bias, H):
    """Gated MHA: queries from xq rows, keys/values from xkv (full length).

    xq: [..., Lq, c], xkv: [..., Lk, c], bias broadcastable to [..., H, Lq, Lk].
    """
    ch = p['wq'].shape[1] // H
    shq = xq.shape[:-1] + (H, ch)
    shk = xkv.shape[:-1] + (H, ch)
    q = (xq @ p['wq']).reshape(shq) * (ch ** -0.5)
    k = (xkv @ p['wk']).reshape(shk)
    v = (xkv @ p['wv']).reshape(shk)
    a = jax.nn.softmax(jnp.einsum('...qhc,...khc->...hqk', q, k) + bias, axis=-1)
    o = jnp.einsum('...hqk,...khc->...qhc', a, v)
    g = jax.nn.sigmoid(xq @ p['wg'] + p['bg']).reshape(shq)
    o = (o * g).reshape(xq.shape[:-1] + (H * ch,))
    return o @ p['wo'] + p['bo']


def _tri_att_local(x, pb, pp):
    """Gated MHA within axis 2 of x [N, B, L, C], per-head (transpose-free).

    pb: [N, L, L, H_P] global pair-bias map; bias[h, q, k] = pb[n, q, k, h].
    Returns the gated, projected update [N, B, L, C_Z].
    """
    ch = CH_PA
    wq3 = pp['wq'].reshape(C_Z, H_P, ch)
    wk3 = pp['wk'].reshape(C_Z, H_P, ch)
    wv3 = pp['wv'].reshape(C_Z, H_P, ch)
    wg3 = pp['wg'].reshape(C_Z, H_P, ch)
    bg3 = pp['bg'].reshape(H_P, ch)
    wo3 = pp['wo'].reshape(H_P, ch, C_Z)
    acc = pp['bo']
    for h in range(H_P):
        qh = (x @ wq3[:, h]) * (ch ** -0.5)                         # [N,B,L,ch]
        kh = x @ wk3[:, h]
        vh = x @ wv3[:, h]
        sc = jnp.einsum('nbqc,nbkc->nbqk', qh, kh) + pb[:, None, :, :, h]
        a = jax.nn.softmax(sc, axis=-1)
        oh = jnp.einsum('nbqk,nbkc->nbqc', a, vh)
        gh = jax.nn.sigmoid(x @ wg3[:, h] + bg3[h])
        acc = acc + (oh * gh) @ wo3[h]
    return acc


def _transition(x, p):
    x = _ln(x, p['ln_g'], p['ln_b'])
    return jax.nn.relu(x @ p['w1'] + p['b1']) @ p['w2'] + p['b2']


def _evoformer_shard(s, z, p):
    """Runs on one core. s: [1, L, C_S] replicated; z: [1, RS, L, C_Z] row shard.

    Returns (s_shard [1, RS, C_S], z_shard [1, RS, L, C_Z]).
    """
    ax = 'x'

    # ---- s track ----
    sn = _ln(s, p['sa']['ln_s_g'], p['sa']['ln_s_b'])
    znb = _ln(z, p['sa']['ln_z_g'], p['sa']['ln_z_b'])
    # bias rows for our z shard -> all-gather to full [N, H_S, L, L]
    bias_sh = jnp.transpose(znb @ p['sa']['w_bias'], (0, 3, 1, 2))  # [N,H,RS,L]
    bias = jax.lax.all_gather(bias_sh, ax, axis=2, tiled=True)      # [N,H,L,L]
    # each core computes attention for its RS query rows of s
    i0 = jax.lax.axis_index(ax) * RS
    sn_q = jax.lax.dynamic_slice_in_dim(sn, i0, RS, axis=1)
    s_q = jax.lax.dynamic_slice_in_dim(s, i0, RS, axis=1)
    bias_q = jax.lax.dynamic_slice_in_dim(bias, i0, RS, axis=2)
    s_row = s_q + _mha_rows(sn_q, sn, p['sa'], bias_q, H_S)         # [N,RS,C_S]
    s_row = s_row + _transition(s_row, p['st'])
    # full s needed for opm bf over all j
    s_full = jax.lax.all_gather(s_row, ax, axis=1, tiled=True)      # [N,L,C_S]

    # ---- outer product mean ----
    snf = _ln(s_full, p['opm']['ln_g'], p['opm']['ln_b'])
    af = jax.lax.dynamic_slice_in_dim(snf, i0, RS, axis=1) @ p['opm']['w1'] + p['opm']['b1']
    bf = snf @ p['opm']['w2'] + p['opm']['b2']                      # [N,L,ch]
    # factored: (af x bf) . wo == af . (bf . wo_abc)  -- rank-32, ~30x fewer MACs
    wo3 = p['opm']['wo'].reshape(CH_OPM, CH_OPM, C_Z)               # [a,b,c]
    V = jnp.einsum('nia,abc->nibc', af, wo3)                        # [N,RS,b,C]
    z = z + jnp.einsum('njb,nibc->nijc', bf, V) + p['opm']['bo']

    # ---- triangle mult outgoing: x[i,j,:] = sum_k a[i,k] b[j,k] ----
    # c-major batching: transpose the small local a/b before the all-gather so
    # the batched GEMM sees leading batch dims (no 19MB transposes).
    pp = p['tmo']
    zn = _ln(z, pp['ln_in_g'], pp['ln_in_b'])
    a = jax.nn.sigmoid(zn @ pp['w_ag'] + pp['b_ag']) * (zn @ pp['w_ap'] + pp['b_ap'])
    b = jax.nn.sigmoid(zn @ pp['w_bg'] + pp['b_bg']) * (zn @ pp['w_bp'] + pp['b_bp'])
    a_t = jnp.transpose(a, (0, 3, 1, 2))                            # [N,C,RS,L]
    b_t = jnp.transpose(b, (0, 3, 1, 2))                            # [N,C,RS,L]
    b_full = jax.lax.all_gather(b_t, ax, axis=2, tiled=True)        # [N,C,L,L]
    x_t = jnp.einsum('ncik,ncjk->ncij', a_t, b_full)                # [N,C,RS,L]
    x = jnp.transpose(x_t, (0, 2, 3, 1))                            # [N,RS,L,C]
    x = _ln(x, pp['ln_out_g'], pp['ln_out_b']) @ pp['w_z'] + pp['b_z']
    z = z + jax.nn.sigmoid(zn @ pp['w_g'] + pp['b_g']) * x

    # ---- triangle mult incoming: x[i,j,:] = sum_k a[k,i] b[k,j] ----
    pp = p['tmi']
    zn = _ln(z, pp['ln_in_g'], pp['ln_in_b'])
    a = jax.nn.sigmoid(zn @ pp['w_ag'] + pp['b_ag']) * (zn @ pp['w_ap'] + pp['b_ap'])
    b = jax.nn.sigmoid(zn @ pp['w_bg'] + pp['b_bg']) * (zn @ pp['w_bp'] + pp['b_bp'])
    a_t = jnp.transpose(a, (0, 3, 1, 2))
    b_t = jnp.transpose(b, (0, 3, 1, 2))
    a_full = jax.lax.all_gather(a_t, ax, axis=2, tiled=True)        # [N,C,L(k),L(i)]
    b_full = jax.lax.all_gather(b_t, ax, axis=2, tiled=True)        # [N,C,L(k),L(j)]
    a_i = jax.lax.dynamic_slice_in_dim(a_full, i0, RS, axis=3)      # [N,C,L,RS]
    x_t = jnp.einsum('ncki,nckj->ncij', a_i, b_full)                # [N,C,RS,L]
    x = jnp.transpose(x_t, (0, 2, 3, 1))
    x = _ln(x, pp['ln_out_g'], pp['ln_out_b']) @ pp['w_z'] + pp['b_z']
    z = z + jax.nn.sigmoid(zn @ pp['w_g'] + pp['b_g']) * x

    # ---- triangle attention starting (rows attend within row) ----
    # einsum-only form: batch (n, i=row), q/k = columns; no big transposes.
    pp = p['tas']
    zn = _ln(z, pp['ln_g'], pp['ln_b'])
    pb_loc = zn @ pp['w_bias']                                      # [N,RS,L,H]
    pb = jax.lax.all_gather(pb_loc, ax, axis=1, tiled=True)         # [N,L(q),L(k),H]
    z = z + _tri_att_local(zn, pb, pp)

    # ---- triangle attention ending (columns attend within column) ----
    # Reshard rows->columns with all_to_all, run the same local attention in
    # the transposed frame, reshard back. No full-tensor gathers/transposes.
    pp = p['tae']
    zn = _ln(z, pp['ln_g'], pp['ln_b'])
    zn_cs = jax.lax.all_to_all(zn, ax, split_axis=2, concat_axis=1, tiled=True)
    # zn_cs: [N, L(row), RS(our cols), C] -> transposed frame [N, RS(col), L(row), C]
    znT = jnp.transpose(zn_cs, (0, 2, 1, 3))
    pb_cs = znT @ pp['w_bias']                                      # [N,RS(col),L(row),H]
    pb_full = jax.lax.all_gather(pb_cs, ax, axis=1, tiled=True)     # [N,L(col=q),L(row=k),H]
    o = _tri_att_local(znT, pb_full, pp)                            # [N,RS(col),L(row),C]
    oT = jnp.transpose(o, (0, 2, 1, 3))                             # [N,L(row),RS(col),C]
    z = z + jax.lax.all_to_all(oT, ax, split_axis=1, concat_axis=2, tiled=True)

    # ---- pair transition ----
    z = z + _transition(z, p['pt'])
    return s_row, z


def _build_sharded(params):
    mesh = Mesh(np.array(jax.devices()[:NCORES]), ('x',))

    @functools.partial(shard_map, mesh=mesh,
                       in_specs=(P(), P(None, 'x', None, None)),
                       out_specs=(P(None, 'x', None), P(None, 'x', None, None)),
                       check_rep=False)
    def f(s, z):
        return _evoformer_shard(s, z, params)

    return jax.jit(f)


def kernel(s, z, params):
    s = jnp.asarray(s); z = jnp.asarray(z)
    f = _build_sharded(params)
    s_out, z_out = f(s, z)
    return np.asarray(s_out), np.asarray(z_out)
